# revision 3
# baseline (speedup 1.0000x reference)
"""DeepSets segment-reduce kernel for 8 Trainium2 NeuronCores (v2).

Math: out[s] = sum_{i in s} (x_i @ W + b) = (sum_{i in s} x_i) @ W + count_s * b.
The device computes per-segment component sums of the 2-dim points plus a
count-weighted bias via one small matmul; the [N, 64] intermediate never
exists.

v2 layout: host zero-pads each segment into a fixed 2L-wide bf16 slot
([x0 x L | x1 x L], segment g*128+p at row p, group-block g), so the device
needs NO mask, NO iota and no over-gather: the gather DMA moves exactly
(padded) segment data at half (bf16) width. Per group the DVE does two
bf16 fold-adds (2x perf mode) to shrink 2L -> L/2, then one f32-out reduce;
folds for group g start while group g+1 is still streaming. A PE transpose
+ one [12x256] block-diagonal matmul (W rows + bias-by-count rows) produce
all 512 output rows per core. Out-DMA is issued from the scalar-engine
queue so it never blocks the next iteration's gathers; gx is
double-buffered so iterations pipeline at max(DMA, DVE) ~= 7us.

DEEPSETS_BENCH_ITERS=k repeats the body k times for wall-clock delta timing.
"""

import os
from contextlib import ExitStack

import numpy as np
import ml_dtypes

import concourse.bass as bass
import concourse.mybir as mybir
from concourse.bass_utils import run_bass_kernel_spmd

P = 128
G = 4
CORES = 8
NUM_SEGMENTS = 4096
SEGC = NUM_SEGMENTS // CORES     # 512
FEAT = 64
META_W = G + 12 * 0 + G * FEAT + P  # cnt(4) + W12(256) + identity(128) = 388

_kernel_cache: dict = {}


def _build(L: int, iters: int) -> bass.Bass:
    """L: padded points per slot (multiple of 4)."""
    H = L // 2
    Q = L // 4
    SLOT = 2 * L              # bf16 elems per slot row (x0 block + x1 block)
    f32 = mybir.dt.float32
    bf16 = mybir.dt.bfloat16
    nc = bass.Bass()

    xsB = nc.dram_tensor("xsB", [P, G * SLOT], bf16, kind="ExternalInput")
    blob = nc.dram_tensor("blob", [P, META_W], f32, kind="ExternalInput")
    outd = nc.dram_tensor("outd", [P, G * FEAT], f32, kind="ExternalOutput")

    with ExitStack() as ctx:
        meta = ctx.enter_context(nc.sbuf_tensor("meta", [P, META_W], f32))
        gx0 = ctx.enter_context(nc.sbuf_tensor("gx0", [P, G * SLOT], bf16))
        gx1 = ctx.enter_context(nc.sbuf_tensor("gx1", [P, G * SLOT], bf16))
        gh1 = ctx.enter_context(nc.sbuf_tensor("gh1", [P, G * 2 * H], bf16))
        gh2 = ctx.enter_context(nc.sbuf_tensor("gh2", [P, G * 2 * Q], bf16))
        sums12 = ctx.enter_context(nc.sbuf_tensor("sums12", [P, 12], f32))
        s3t = ctx.enter_context(nc.sbuf_tensor("s3t", [12, P], bf16))
        w12b = ctx.enter_context(nc.sbuf_tensor("w12b", [12, G * FEAT], bf16))
        outb = ctx.enter_context(nc.sbuf_tensor("outb", [P, G * FEAT], f32))
        psum12 = ctx.enter_context(nc.psum_tensor("psum12", [12, P], f32))
        pso = ctx.enter_context(nc.psum_tensor("pso", [P, G * FEAT], f32))
        bsem = ctx.enter_context(nc.semaphore("bsem"))
        gsem = ctx.enter_context(nc.semaphore("gsem"))
        rsem = ctx.enter_context(nc.semaphore("rsem"))
        s3sem = ctx.enter_context(nc.semaphore("s3sem"))
        csem = ctx.enter_context(nc.semaphore("csem"))
        pesem = ctx.enter_context(nc.semaphore("pesem"))
        osem = ctx.enter_context(nc.semaphore("osem"))
        block = ctx.enter_context(nc.Block())

        gxs = [gx0, gx1]
        w12_ap = meta[0:12, G:G + G * FEAT]
        ident_ap = meta[:, G + G * FEAT:META_W]

        def fold_aps(src, src_blk, dst, dst_blk, width, g):
            """in0/in1 = first/second half of each component block; out packed."""
            i0 = bass.AP(tensor=src[:, :].tensor, offset=g * src_blk * 2,
                         ap=[[G * src_blk * 2, P], [src_blk, 2], [1, width]])
            i1 = bass.AP(tensor=src[:, :].tensor, offset=g * src_blk * 2 + width,
                         ap=[[G * src_blk * 2, P], [src_blk, 2], [1, width]])
            o = bass.AP(tensor=dst[:, :].tensor, offset=g * width * 2,
                        ap=[[G * width * 2, P], [width, 2], [1, width]])
            return i0, i1, o

        @block.sync
        def _(sync):
            for it in range(iters):
                if it >= 2:
                    # gx[it%2] was last read by iter it-2's folds (reduce g3
                    # of it-2 follows them in DVE order).
                    sync.wait_ge(rsem, 4 * (it - 1))
                buf = gxs[it % 2]
                for g in range(G):
                    sync.dma_start(
                        buf[:, g * SLOT:(g + 1) * SLOT],
                        xsB[:, g * SLOT:(g + 1) * SLOT],
                    ).then_inc(gsem, 16)

        @block.scalar
        def _(scalar):
            # blob load, one-time counts copy, and the whole PSUM-drain tail
            # live on the otherwise-idle Activation engine so DVE stays a
            # pure fold/reduce pipeline.
            copy = mybir.ActivationFunctionType.Copy
            scalar.dma_start(meta[:, :], blob[:, :]).then_inc(bsem, 16)
            scalar.wait_ge(bsem, 16)
            nc.scalar.activation(
                out=sums12[:, 8:12], in_=meta[:, 0:G], func=copy,
            ).then_inc(csem, 1)
            nc.scalar.activation(out=w12b[:, :], in_=w12_ap, func=copy)
            for it in range(iters):
                scalar.wait_ge(pesem, 2 * it + 1)
                nc.scalar.activation(
                    out=s3t[:, :], in_=psum12[:, :], func=copy,
                ).then_inc(s3sem, 1)
                scalar.wait_ge(pesem, 2 * it + 2)
                if it >= 1:
                    scalar.wait_ge(osem, 16 * it)
                nc.scalar.activation(out=outb[:, :], in_=pso[:, :], func=copy)
                scalar.dma_start(outd[:, :], outb[:, :]).then_inc(osem, 16)

        @block.vector
        def _(vector):
            for it in range(iters):
                buf = gxs[it % 2]
                with nc.allow_low_precision(reason="bf16 folds; gate is 2e-2"):
                    for g in range(G):
                        vector.wait_ge(gsem, 16 * (4 * it + g + 1))
                        if g == 0 and it >= 1:
                            # transpose(it-1) must read sums12 first
                            vector.wait_ge(pesem, 2 * (it - 1) + 1)
                        i0, i1, o = fold_aps(buf, L, gh1, H, H, g)
                        nc.vector.tensor_tensor(
                            out=o, in0=i0, in1=i1, op=mybir.AluOpType.add)
                        i0, i1, o = fold_aps(gh1, H, gh2, Q, Q, g)
                        nc.vector.tensor_tensor(
                            out=o, in0=i0, in1=i1, op=mybir.AluOpType.add)
                        red_in = bass.AP(
                            tensor=gh2[:, :].tensor, offset=g * 2 * Q,
                            ap=[[G * 2 * Q, P], [Q, 2], [1, Q]])
                        red_out = bass.AP(
                            tensor=sums12[:, :].tensor, offset=2 * g,
                            ap=[[12, P], [1, 2]])
                        nc.vector.reduce_sum(
                            out=red_out, in_=red_in, axis=mybir.AxisListType.X,
                        ).then_inc(rsem, 1)

        @block.tensor
        def _(tensor):
            for it in range(iters):
                tensor.wait_ge(rsem, 4 * (it + 1))
                if it == 0:
                    tensor.wait_ge(csem, 1)
                nc.tensor.transpose(
                    out=psum12[:, :], in_=sums12[:, :], identity=ident_ap,
                ).then_inc(pesem, 1)
                tensor.wait_ge(s3sem, it + 1)
                nc.tensor.matmul(
                    out=pso[:, :], lhsT=s3t[:, :], rhs=w12b[:, :],
                    start=True, stop=True,
                ).then_inc(pesem, 1)

    return nc


def _get_kernel(L: int, iters: int) -> bass.Bass:
    key = (L, iters)
    if key not in _kernel_cache:
        _kernel_cache[key] = _build(L, iters)
    return _kernel_cache[key]


def kernel(x, segment_ids, W, b, num_segments, **_unused):
    x = np.ascontiguousarray(np.asarray(x, dtype=np.float32))
    ids = np.asarray(segment_ids)
    W = np.asarray(W, dtype=np.float32)
    b = np.asarray(b, dtype=np.float32)
    S = int(num_segments)
    assert S == NUM_SEGMENTS, f"kernel hardcoded for {NUM_SEGMENTS} segments"
    N = x.shape[0]
    iters = int(os.environ.get("DEEPSETS_BENCH_ITERS", "1"))

    bounds = np.searchsorted(ids, np.arange(S + 1), side="left").astype(np.int64)
    lens = np.diff(bounds)
    L = int(((lens.max() + 3) // 4) * 4)
    SLOT = 2 * L

    nc = _get_kernel(L, iters)

    # Zero-padded slot slab, all segments at once: [S, L] per component.
    idx = bounds[:-1, None] + np.arange(L)[None, :]          # [S, L]
    valid = np.arange(L)[None, :] < lens[:, None]
    idxc = np.minimum(idx, N - 1)
    x0 = np.where(valid, x[idxc, 0], 0.0).astype(ml_dtypes.bfloat16)
    x1 = np.where(valid, x[idxc, 1], 0.0).astype(ml_dtypes.bfloat16)

    # W12 block-diagonal [12, 256]: rows 2g+c -> W[c], rows 8+g -> b
    w12 = np.zeros((12, G * FEAT), np.float32)
    for g in range(G):
        for c2 in range(2):
            w12[2 * g + c2, g * FEAT:(g + 1) * FEAT] = W[c2]
        w12[8 + g, g * FEAT:(g + 1) * FEAT] = b
    ident = np.eye(P, dtype=np.float32)

    in_maps = []
    for c in range(CORES):
        seg0 = c * SEGC
        # xsB[p, g*SLOT + cmp*L + j] = component cmp of point j of segment
        # seg0 + g*128 + p
        xsB = np.empty((P, G, 2, L), ml_dtypes.bfloat16)
        segs = (seg0 + np.arange(SEGC)).reshape(G, P)        # [G, P]
        xsB[:, :, 0, :] = x0[segs].transpose(1, 0, 2)
        xsB[:, :, 1, :] = x1[segs].transpose(1, 0, 2)
        blobv = np.zeros((P, META_W), np.float32)
        blobv[:, 0:G] = lens[seg0:seg0 + SEGC].reshape(G, P).T
        blobv[0:12, G:G + G * FEAT] = w12
        blobv[:, G + G * FEAT:META_W] = ident
        in_maps.append({"xsB": xsB.reshape(P, G * SLOT), "blob": blobv})

    res = run_bass_kernel_spmd(nc, in_maps, core_ids=list(range(CORES)))
    parts = [
        res.results[c]["outd"].reshape(P, G, FEAT).transpose(1, 0, 2).reshape(
            SEGC, FEAT
        )
        for c in range(CORES)
    ]
    return np.concatenate(parts, axis=0).astype(np.float32)
